# revision 14
# baseline (speedup 1.0000x reference)
"""Trainium2 Bass kernel for nn_GaussianDistribution, v2.

Design (cost-model-driven):
- Phase A (cov build): J = n2*R quaternion algebra. f32 through the
  cancellation-sensitive part (squares/sums/n2/J), bf16 for the rank-2
  assembly (w/m/plane sums). cov kept as the 6 unique planes of the
  symmetric 3x3, bf16. Chunk 0 biased to DVE (fills its startup idle),
  chunk 1 biased to Pool (runs under the phase-B stream), emission
  interleaved between batches to respect in-order engine streams.
- Phase B in bf16: Act splits eps into dense per-i planes and (with Pool)
  expands cov planes over the S=8 sample dim, so every DVE tensor_tensor
  runs in 2x_1p mode (bf16, unit stride). Accumulate split DVE/Pool,
  clip on DVE (bf16 tensor_scalar 4x), final mean-add (bf16 + f32 mean
  -> f32 out) on Pool.
- prep(b+1) emitted before compute(b) so Pool's cexp never queues behind
  the previous batch's final add.

Sharding: pure data parallel over batch (B=64 -> 8 per core).
"""
import sys

sys.path.insert(0, "/opt/trn_rl_repo")
from contextlib import ExitStack

import numpy as np

import concourse.bass as bass
import concourse.tile as tile
from concourse import masks
from concourse import mybir

AF = mybir.ActivationFunctionType
OP = mybir.AluOpType
F32 = mybir.dt.float32
BF16 = mybir.dt.bfloat16

NCORES = 8
B, N, S = 64, 16384, 8
BPC = B // NCORES          # batches per core (8)
P = 128                    # partitions
FPB = N // P               # points per partition per batch (128)
NCH = 2                    # phase-A chunks
BPCH = BPC // NCH          # batches per chunk (4)
CHF = BPCH * FPB           # free dim per chunk plane (512)
TOTF = BPC * FPB           # free dim of persistent planes (1024)


import json as _json
import os as _os


def _knob(name, default):
    v = _os.environ.get(name)
    return default if v is None else _json.loads(v)


NQ = _knob('K_NQ', 2)      # eps load chunks per batch
FQ = FPB // NQ             # points per chunk

# --- engine assignment knobs -------------------------------------------
# phase-A group -> engine per chunk ("V" = DVE, "P" = Pool)
A_ENG = [
    {"sums": "P", "jdiag": "P", "prods": "V", "joff": "P",
     "b": "V", "bt": "V", "w": "V", "m": "P", "madd": "V",
     **_knob("K_A0", {})},                                    # chunk 0
    {"sums": "P", "jdiag": "P", "prods": "P", "joff": "P",
     "b": "V", "bt": "V", "w": "P", "m": "P", "madd": "P",
     **_knob("K_A1", {})},                                    # chunk 1
]
POOL_CEXP = _knob('K_POOL_CEXP', 2)
DVE_CEXP = {int(k): v for k, v in _knob('K_DVE_CEXP', {"0": 3, "1": 3}).items()}
POOL_ADD2_BATCHES = tuple(_knob('K_POOL_ADD2', [1, 2, 3, 4, 5, 6]))
POOL_MUL_BATCHES = tuple(_knob('K_POOL_MUL', [6, 7]))
NH = {int(k): v for k, v in _knob('K_NH', {"0": 2, "6": 4, "7": 4}).items()}


def split_excess_waits(nc, limits={}, default_limit=1):
    """Walrus rejects >1 sem-wait on Drain (cap others at the limit). Move
    excess immediate waits onto standalone InstEventSemaphore before."""
    ctr = 0
    for blk in nc.m.functions[0].blocks:
        new = []
        for inst in blk.instructions:
            si = inst.sync_info
            waits = list(si.on_wait) if (si and si.on_wait) else []
            lim = limits.get(type(inst).__name__, default_limit)
            if len(waits) > lim:
                reg = [w for w in waits if w.wait_reg is not None]
                imm = [w for w in waits if w.wait_reg is None]
                ordered = reg + imm
                keep, excess = ordered[:lim], ordered[lim:]
                assert all(w.wait_reg is None for w in excess)
                for w in excess:
                    ctr += 1
                    new.append(mybir.InstEventSemaphore(
                        name=f"I-waitsplit-{ctr}", engine=inst.engine,
                        sync_info=mybir.SyncInfo(on_wait=[w], on_update=[])))
                si.on_wait = keep
            new.append(inst)
        blk.instructions = new
    return ctr


def _bcast_s(ap):
    """[p, ...] -> [p, ..., S] with 0-step sample dim."""
    return ap.broadcast_to(list(ap.shape) + [S])


# cov plane index for (i, j) in the 6-plane symmetric layout
PLANE = {(0, 0): 0, (1, 1): 1, (2, 2): 2,
         (0, 1): 3, (1, 0): 3, (0, 2): 4, (2, 0): 4, (1, 2): 5, (2, 1): 5}


def build(split=True):
    nc = bass.Bass()
    mean_s = nc.dram_tensor("mean", [BPC, 3, N], F32, kind="ExternalInput")
    scale_s = nc.dram_tensor("scale", [BPC, 3, N], F32, kind="ExternalInput")
    rot_s = nc.dram_tensor("rot", [BPC, 4, N], F32, kind="ExternalInput")
    eps_s = nc.dram_tensor("eps", [BPC * N, S, 3], F32, kind="ExternalInput")
    out_s = nc.dram_tensor("out", [BPC, 3, N * S], F32, kind="ExternalOutput")

    with tile.TileContext(nc) as tc, ExitStack() as ctx:
        io = ctx.enter_context(tc.tile_pool(name="io", bufs=1))
        pa = ctx.enter_context(tc.tile_pool(name="pa", bufs=1))
        cons = ctx.enter_context(tc.tile_pool(name="cons", bufs=1))
        epsp = ctx.enter_context(tc.tile_pool(name="epsp", bufs=_knob("K_EPSPB", 3)))
        ep = ctx.enter_context(tc.tile_pool(name="ep", bufs=_knob("K_EPB", 2)))
        cxp = ctx.enter_context(tc.tile_pool(name="cxp", bufs=_knob("K_CXPB", 2)))
        mp = ctx.enter_context(tc.tile_pool(name="mp", bufs=1))
        vp = ctx.enter_context(tc.tile_pool(name="vp", bufs=2))
        outp = ctx.enter_context(tc.tile_pool(name="outp", bufs=_knob("K_OUTPB", 2)))
        psp = ctx.enter_context(tc.tile_pool(name="psp", bufs=_knob("K_PSPB", 2),
                                             space="PSUM"))

        warm = cons.tile([P, 1], F32, tag="warm", name="warm")
        nc.vector.memset(warm, 0.0)
        nc.scalar.square(warm, warm)
        ident = cons.tile([P, P], BF16, tag="ident", name="ident")
        masks.make_identity(nc, ident[:, :])

        cov_t = cons.tile([P, 6, TOTF], BF16, tag="cov", name="cov_t")
        mean_t = cons.tile([P, 3, TOTF], F32, tag="mean", name="mean_t")

        def eng_of(code):
            return nc.gpsimd if code == "P" else nc.vector

        # ---------------- phase A (generator; yields at segment breaks) ----
        def phase_a_chunk(c):
            cfg = A_ENG[c]
            bsl = slice(c * BPCH, (c + 1) * BPCH)
            sl = slice(c * CHF, (c + 1) * CHF)

            def tt(group, fn, *args):
                getattr(eng_of(cfg[group]), f"tensor_{fn}")(*args)

            def load_plane(tag, src, eng=None):
                t = io.tile([P, CHF], F32, tag=tag, name=tag)
                (eng or nc.sync).dma_start(
                    out=t[:, :].rearrange("p (b f) -> p b f", f=FPB),
                    in_=src.rearrange("b (p f) -> p b f", p=P))
                return t
            sc_eng = ({"SP": nc.sync, "A": nc.scalar, "P": nc.gpsimd}[
                _knob("K_SC_ENG", "SP")] if c == 0 else None)

            r = load_plane("rot_r", rot_s[bsl, 0, :])
            x = load_plane("rot_x", rot_s[bsl, 1, :])
            y = load_plane("rot_y", rot_s[bsl, 2, :])
            z = load_plane("rot_z", rot_s[bsl, 3, :])
            s0 = load_plane("sc_0", scale_s[bsl, 0, :], sc_eng)
            s1 = load_plane("sc_1", scale_s[bsl, 1, :], sc_eng)
            s2 = load_plane("sc_2", scale_s[bsl, 2, :], sc_eng)

            def Tf(tag):
                return pa.tile([P, CHF], F32, tag=tag, name=tag)

            def Tb(tag):
                return pa.tile([P, CHF], BF16, tag=tag, name=tag)

            # Act: interleaved doubles/squares for earliest availability
            r2, x2, y2, z2 = Tf("fa"), Tf("fb"), Tf("fc"), Tf("fd")
            xt, yt, zt = Tf("fe"), Tf("ff_"), Tf("fg")
            nc.scalar.square(r2, r)
            nc.scalar.activation(xt, x, AF.Copy, scale=2.0)
            nc.scalar.square(x2, x)
            nc.scalar.activation(yt, y, AF.Copy, scale=2.0)
            nc.scalar.square(y2, y)
            nc.scalar.activation(zt, z, AF.Copy, scale=2.0)
            nc.scalar.square(z2, z)
            s0q, s1q, s2q = Tb("s0q"), Tb("s1q"), Tb("s2q")
            nc.scalar.square(s0q, s0)
            nc.scalar.square(s1q, s1)
            nc.scalar.square(s2q, s2)

            # pair products first: they head the DVE stream in chunk 0
            xy2, xz2, yz2 = Tf("pa_"), Tf("pb_"), Tf("pc_")
            rx2, ry2, rz2 = Tf("pd_"), Tf("pe_"), Tf("pf_")
            tt("prods", "mul", xy2, xt, y)
            tt("prods", "mul", xz2, xt, z)
            tt("prods", "mul", yz2, yt, z)
            tt("prods", "mul", rx2, r, xt)
            tt("prods", "mul", ry2, r, yt)
            tt("prods", "mul", rz2, r, zt)

            # sums (6) + n2 (f32)
            aa, bb, cc = Tf("fh"), Tf("fi"), Tf("fj")
            dd, ee, ff = Tf("fk"), Tf("fl"), Tf("fm")
            tt("sums", "add", aa, r2, x2)
            tt("sums", "add", bb, y2, z2)
            tt("sums", "add", cc, r2, y2)
            tt("sums", "add", dd, x2, z2)
            tt("sums", "add", ee, r2, z2)
            tt("sums", "add", ff, x2, y2)
            n2 = Tf("fn")
            nc.vector.tensor_add(n2, aa, bb)
            rr = Tf("fo")
            nc.vector.reciprocal(rr, n2)
            iv = Tb("iv")
            nc.scalar.square(iv, rr)

            yield

            # J planes (f32 compute, bf16 store) — reuse dead f32 slots
            jtag = {0: "fa", 4: "fb", 8: "fc",
                    1: "fd", 3: "fh", 2: "fi", 6: "fj", 5: "fk", 7: "fl"}
            J = {}
            for idx, (u, v_) in zip((0, 4, 8), ((aa, bb), (cc, dd), (ee, ff))):
                J[idx] = Tb(jtag[idx])
                tt("jdiag", "sub", J[idx], u, v_)

            for idx, fn, (u, v_) in (
                    (1, "sub", (xy2, rz2)), (3, "add", (xy2, rz2)),
                    (2, "add", (xz2, ry2)), (6, "sub", (xz2, ry2)),
                    (5, "sub", (yz2, rx2)), (7, "add", (yz2, rx2))):
                J[idx] = Tb(jtag[idx])
                tt("joff", fn, J[idx], u, v_)

            yield

            b0, b1 = Tb("b0"), Tb("b1")
            tt("b", "sub", b0, s0q, s2q)
            tt("b", "sub", b1, s1q, s2q)
            bt0, bt1 = Tb("bt0"), Tb("bt1")
            tt("bt", "mul", bt0, b0, iv)
            tt("bt", "mul", bt1, b1, iv)

            wtag = ["s0q", "s1q", "b0", "b1", "fm", "fn"]
            w = {}
            for i in range(3):
                for k in range(2):
                    w[(i, k)] = Tb(wtag[i * 2 + k])
                    tt("w", "mul", w[(i, k)], bt0 if k == 0 else bt1,
                       J[i * 3 + k])

            yield

            # cov planes: cov_ij = w_i0*J_j0 + w_i1*J_j1 (+ s2q if i==j)
            for e, (i, j) in enumerate([(0, 0), (1, 1), (2, 2),
                                        (0, 1), (0, 2), (1, 2)]):
                m1 = pa.tile([P, CHF], BF16, tag=("iv", "bt0")[e % 2],
                             name="am1")
                m2 = pa.tile([P, CHF], BF16, tag=("bt1", "am2x")[e % 2],
                             name="am2")
                tt("m", "mul", m1, w[(i, 0)], J[j * 3 + 0])
                tt("m", "mul", m2, w[(i, 1)], J[j * 3 + 1])
                dst = cov_t[:, PLANE[(i, j)], sl]
                if i == j:
                    tt("madd", "add", dst, m1, m2)
                    tt("madd", "add", dst, dst, s2q)
                else:
                    tt("madd", "add", dst, m1, m2)

            yield

        # ---------------- phase B ------------------------------------------
        e_ts, cx_ts = {}, {}

        def prep(b):
            """eps DMA + split (Act), cov s-expansion (Act/Pool) for batch b."""
            bsl = slice(b * FPB, (b + 1) * FPB)
            e_t = ep.tile([P, 3, FPB * S], BF16, tag="e", name="e_t")
            e_ts[b] = e_t
            EPS_ENG = _knob("K_EPS_ENG", {})
            for q in range(NQ):
                eq = epsp.tile([P, FQ * S * 3], F32, tag="epsq", name="eq")
                engs = EPS_ENG.get(str(b), ["SP"] * NQ)
                deng = {"SP": nc.sync, "A": nc.scalar,
                        "P": nc.gpsimd}[engs[q % len(engs)]]
                deng.dma_start(
                    out=eq[:, :],
                    in_=eps_s[b * N:(b + 1) * N, :, :].rearrange(
                        "(p f) s i -> p (f s i)", p=P)[
                        :, q * FQ * S * 3:(q + 1) * FQ * S * 3])
                if (q in _knob("K_POOL_ESPLIT_Q", []) or
                        b in _knob("K_POOL_ESPLIT_B", [])):
                    nc.gpsimd.tensor_copy(
                        e_t[:, :, q * FQ * S:(q + 1) * FQ * S],
                        eq[:, :].rearrange("p (f s i) -> p i (f s)", s=S, i=3))
                elif b in _knob("K_DVE_ESPLIT", [3, 4, 5, 6, 7]):
                    nc.vector.tensor_copy(
                        e_t[:, :, q * FQ * S:(q + 1) * FQ * S],
                        eq[:, :].rearrange("p (f s i) -> p i (f s)", s=S, i=3))
                else:
                    nc.scalar.activation(
                        e_t[:, :, q * FQ * S:(q + 1) * FQ * S],
                        eq[:, :].rearrange("p (f s i) -> p i (f s)", s=S, i=3),
                        AF.Copy)

            cx = cxp.tile([P, 6, FPB, S], BF16, tag="cx", name="cx")
            cx_ts[b] = cx
            nd = DVE_CEXP.get(b, 0)
            for u in range(POOL_CEXP):
                nc.gpsimd.tensor_copy(cx[:, u], _bcast_s(cov_t[:, u, bsl]))
            for u in range(POOL_CEXP, POOL_CEXP + nd):
                nc.vector.tensor_copy(cx[:, u], _bcast_s(cov_t[:, u, bsl]))
            u0 = POOL_CEXP + nd
            while u0 < 6:
                u1 = min(u0 + 2, 6)   # pairs: amortize init, keep granularity
                nc.scalar.activation(
                    cx[:, u0:u1], _bcast_s(cov_t[:, u0:u1, bsl]), AF.Copy)
                u0 = u1

        out_ts = {}

        OUT_ENG = _knob("K_OUT_ENG", {"6": ["SP", "A"], "7": ["SP", "A"]})

        def emit_out(b):
            out_t, nh = out_ts.pop(b)
            engs = OUT_ENG.get(str(b), ["SP"] * nh)
            ob = out_s[b, :, :].rearrange("j (p f) -> p j f", p=P)
            fh = FPB * S // nh
            for h in range(nh):
                fsl = slice(h * fh, (h + 1) * fh)
                eng = {"SP": nc.sync, "A": nc.scalar,
                       "P": nc.gpsimd}[engs[h % len(engs)]]
                eng.dma_start(out=ob[:, :, fsl], in_=out_t[:, :, fsl])

        def compute(b):
            e_t, cx = e_ts.pop(b), cx_ts.pop(b)

            m = [mp.tile([P, 3, FPB * S], BF16, tag=f"m{i}", name=f"m{i}")
                 for i in range(3)]
            v = vp.tile([P, 3, FPB * S], BF16, tag="v", name="v")
            out_t = outp.tile([P, 3, FPB * S], F32, tag="out", name="out_t")
            nh = max(NH.get(b, 1), 2)
            out_ts[b] = (out_t, nh)
            fh = FPB // nh
            for h in range(nh):
                fsl = slice(h * fh * S, (h + 1) * fh * S)      # (f s) slice
                csl = slice(h * fh, (h + 1) * fh)              # f slice
                bsl = slice(b * FPB + h * fh, b * FPB + (h + 1) * fh)
                MORDER = _knob("K_MORDER", [[0, 0], [1, 1], [0, 1], [1, 0],
                                            [2, 2], [0, 2], [1, 2], [2, 0],
                                            [2, 1]])
                for i, j in MORDER:
                    eng = (nc.gpsimd if
                           (i == 2 and b in POOL_MUL_BATCHES) or
                           (i == 2 and j == 2 and
                            b in _knob("K_POOL_MUL1", []))
                           else nc.vector)
                    eng.tensor_mul(
                        m[i][:, j, fsl],
                        e_t[:, i, fsl],
                        cx[:, PLANE[(i, j)], csl].rearrange(
                            "p f s -> p (f s)"))

                # PE: v_psum = m0 + m1 + m2 via identity-matmul accumulation
                # (one PSUM bank per j-plane; 512 f32 each)
                ps = psp.tile([P, 3, fh * S], F32, tag="ps", name="ps")
                for j in range(3):
                    for i in range(3):
                        nc.tensor.matmul(
                            ps[:, j, :], ident[:, :], m[i][:, j, fsl],
                            start=(i == 0), stop=(i == 2))
                vh = v[:, :, fsl]
                if b in _knob("K_FUSE_CLIP", []):
                    # fused PSUM drain + clip on DVE (1x, but one hop less)
                    nc.vector.tensor_scalar(
                        out=vh, in0=ps[:, :, :],
                        scalar1=1.0, scalar2=-1.0, op0=OP.min, op1=OP.max)
                else:
                    # Act: PSUM -> SBUF, f32 -> bf16
                    nc.scalar.activation(vh, ps[:, :, :], AF.Copy)
                    # DVE: clip in place (bf16 tensor_scalar, 4x)
                    nc.vector.tensor_scalar(
                        out=vh, in0=vh,
                        scalar1=1.0, scalar2=-1.0, op0=OP.min, op1=OP.max)
                # + mean, f32 out (Pool unless overridden)
                o4 = out_t[:, :, fsl].rearrange("p j (f s) -> p j f s", s=S)
                c4 = vh.rearrange("p j (f s) -> p j f s", s=S)
                feng = (nc.vector if b in _knob("K_DVE_FINAL", [])
                        or (b in _knob("K_ALT_FINAL", []) and h % 2)
                        else nc.gpsimd)
                feng.tensor_add(o4, c4, _bcast_s(mean_t[:, :, bsl]))

        # ---------------- emission schedule --------------------------------
        a0 = phase_a_chunk(0)
        for _ in a0:
            pass
        prep(0)
        # mean loads on the Act queue (v1 cost model: DMA occupies the
        # issuing engine, so SP and Act transfers run concurrently)
        mengs = _knob("K_MEAN_ENG", ["P", "P", "P"])
        for j in range(3):
            meng = {"SP": nc.sync, "A": nc.scalar,
                    "P": nc.gpsimd}[mengs[j % len(mengs)]]
            meng.dma_start(
                out=mean_t[:, j, :].rearrange("p (b f) -> p b f", f=FPB),
                in_=mean_s[:, j, :].rearrange("b (p f) -> p b f", p=P))
        odl = _knob("K_OUTDELAY", 2)
        pend = []

        def flush(keep):
            while len(pend) > keep:
                emit_out(pend.pop(0))

        a1 = phase_a_chunk(1)
        next(a1)
        prep(1)
        next(a1)
        compute(0)
        pend.append(0)
        prep(2)
        next(a1)
        flush(odl)
        compute(1)
        pend.append(1)
        prep(3)
        for _ in a1:
            pass
        for b in range(2, BPC):
            if b + 2 < BPC:
                prep(b + 2)
            flush(odl)
            compute(b)
            pend.append(b)
        flush(0)

    if split:
        split_excess_waits(nc)
    return nc


_NC = None


def kernel(mean, scale, rot, epsilon, num_samples):
    global _NC
    assert int(num_samples) == S
    mean = np.asarray(mean, dtype=np.float32)
    scale = np.asarray(scale, dtype=np.float32)
    rot = np.asarray(rot, dtype=np.float32)
    epsilon = np.asarray(epsilon, dtype=np.float32)
    if _NC is None:
        _NC = build()
    from concourse.bass_utils import run_bass_kernel_spmd
    in_maps = []
    for c in range(NCORES):
        bs = slice(c * BPC, (c + 1) * BPC)
        in_maps.append({
            "mean": np.ascontiguousarray(mean[bs]),
            "scale": np.ascontiguousarray(scale[bs]),
            "rot": np.ascontiguousarray(rot[bs]),
            "eps": np.ascontiguousarray(epsilon[c * BPC * N:(c + 1) * BPC * N]),
        })
    res = run_bass_kernel_spmd(_NC, in_maps, core_ids=list(range(NCORES)))
    return np.concatenate([res.results[i]["out"] for i in range(NCORES)], axis=0)


# revision 15
# speedup vs baseline: 1.0190x; 1.0190x over previous
"""Trainium2 Bass kernel for nn_GaussianDistribution, v2.

Design (cost-model-driven):
- Phase A (cov build): J = n2*R quaternion algebra. f32 through the
  cancellation-sensitive part (squares/sums/n2/J), bf16 for the rank-2
  assembly (w/m/plane sums). cov kept as the 6 unique planes of the
  symmetric 3x3, bf16. Chunk 0 biased to DVE (fills its startup idle),
  chunk 1 biased to Pool (runs under the phase-B stream), emission
  interleaved between batches to respect in-order engine streams.
- Phase B in bf16: Act splits eps into dense per-i planes and (with Pool)
  expands cov planes over the S=8 sample dim, so every DVE tensor_tensor
  runs in 2x_1p mode (bf16, unit stride). Accumulate split DVE/Pool,
  clip on DVE (bf16 tensor_scalar 4x), final mean-add (bf16 + f32 mean
  -> f32 out) on Pool.
- prep(b+1) emitted before compute(b) so Pool's cexp never queues behind
  the previous batch's final add.

Sharding: pure data parallel over batch (B=64 -> 8 per core).
"""
import sys

sys.path.insert(0, "/opt/trn_rl_repo")
from contextlib import ExitStack

import numpy as np

import concourse.bass as bass
import concourse.tile as tile
from concourse import masks
from concourse import mybir

AF = mybir.ActivationFunctionType
OP = mybir.AluOpType
F32 = mybir.dt.float32
BF16 = mybir.dt.bfloat16

NCORES = 8
B, N, S = 64, 16384, 8
BPC = B // NCORES          # batches per core (8)
P = 128                    # partitions
FPB = N // P               # points per partition per batch (128)
NCH = 2                    # phase-A chunks
BPCH = BPC // NCH          # batches per chunk (4)
CHF = BPCH * FPB           # free dim per chunk plane (512)
TOTF = BPC * FPB           # free dim of persistent planes (1024)


import json as _json
import os as _os


def _knob(name, default):
    v = _os.environ.get(name)
    return default if v is None else _json.loads(v)


NQ = _knob('K_NQ', 2)      # eps load chunks per batch
FQ = FPB // NQ             # points per chunk

# --- engine assignment knobs -------------------------------------------
# phase-A group -> engine per chunk ("V" = DVE, "P" = Pool)
A_ENG = [
    {"sums": "P", "jdiag": "P", "prods": "V", "joff": "P",
     "b": "V", "bt": "V", "w": "V", "m": "P", "madd": "V",
     **_knob("K_A0", {})},                                    # chunk 0
    {"sums": "P", "jdiag": "P", "prods": "P", "joff": "P",
     "b": "V", "bt": "V", "w": "P", "m": "P", "madd": "P",
     **_knob("K_A1", {})},                                    # chunk 1
]
POOL_CEXP = _knob('K_POOL_CEXP', 2)
DVE_CEXP = {int(k): v for k, v in _knob('K_DVE_CEXP', {"0": 3, "1": 3}).items()}
POOL_ADD2_BATCHES = tuple(_knob('K_POOL_ADD2', [1, 2, 3, 4, 5, 6]))
POOL_MUL_BATCHES = tuple(_knob('K_POOL_MUL', [5, 6, 7]))
NH = {int(k): v for k, v in _knob('K_NH', {"0": 2, "6": 4, "7": 4}).items()}


def split_excess_waits(nc, limits={}, default_limit=1):
    """Walrus rejects >1 sem-wait on Drain (cap others at the limit). Move
    excess immediate waits onto standalone InstEventSemaphore before."""
    ctr = 0
    for blk in nc.m.functions[0].blocks:
        new = []
        for inst in blk.instructions:
            si = inst.sync_info
            waits = list(si.on_wait) if (si and si.on_wait) else []
            lim = limits.get(type(inst).__name__, default_limit)
            if len(waits) > lim:
                reg = [w for w in waits if w.wait_reg is not None]
                imm = [w for w in waits if w.wait_reg is None]
                ordered = reg + imm
                keep, excess = ordered[:lim], ordered[lim:]
                assert all(w.wait_reg is None for w in excess)
                for w in excess:
                    ctr += 1
                    new.append(mybir.InstEventSemaphore(
                        name=f"I-waitsplit-{ctr}", engine=inst.engine,
                        sync_info=mybir.SyncInfo(on_wait=[w], on_update=[])))
                si.on_wait = keep
            new.append(inst)
        blk.instructions = new
    return ctr


def _bcast_s(ap):
    """[p, ...] -> [p, ..., S] with 0-step sample dim."""
    return ap.broadcast_to(list(ap.shape) + [S])


# cov plane index for (i, j) in the 6-plane symmetric layout
PLANE = {(0, 0): 0, (1, 1): 1, (2, 2): 2,
         (0, 1): 3, (1, 0): 3, (0, 2): 4, (2, 0): 4, (1, 2): 5, (2, 1): 5}


def build(split=True):
    nc = bass.Bass()
    mean_s = nc.dram_tensor("mean", [BPC, 3, N], F32, kind="ExternalInput")
    scale_s = nc.dram_tensor("scale", [BPC, 3, N], F32, kind="ExternalInput")
    rot_s = nc.dram_tensor("rot", [BPC, 4, N], F32, kind="ExternalInput")
    eps_s = nc.dram_tensor("eps", [BPC * N, S, 3], F32, kind="ExternalInput")
    out_s = nc.dram_tensor("out", [BPC, 3, N * S], F32, kind="ExternalOutput")

    with tile.TileContext(nc) as tc, ExitStack() as ctx:
        io = ctx.enter_context(tc.tile_pool(name="io", bufs=1))
        pa = ctx.enter_context(tc.tile_pool(name="pa", bufs=1))
        cons = ctx.enter_context(tc.tile_pool(name="cons", bufs=1))
        epsp = ctx.enter_context(tc.tile_pool(name="epsp", bufs=_knob("K_EPSPB", 3)))
        ep = ctx.enter_context(tc.tile_pool(name="ep", bufs=_knob("K_EPB", 2)))
        cxp = ctx.enter_context(tc.tile_pool(name="cxp", bufs=_knob("K_CXPB", 2)))
        mp = ctx.enter_context(tc.tile_pool(name="mp", bufs=1))
        vp = ctx.enter_context(tc.tile_pool(name="vp", bufs=2))
        outp = ctx.enter_context(tc.tile_pool(name="outp", bufs=_knob("K_OUTPB", 2)))
        psp = ctx.enter_context(tc.tile_pool(name="psp", bufs=_knob("K_PSPB", 2),
                                             space="PSUM"))

        warm = cons.tile([P, 1], F32, tag="warm", name="warm")
        nc.vector.memset(warm, 0.0)
        nc.scalar.square(warm, warm)
        ident = cons.tile([P, P], BF16, tag="ident", name="ident")
        masks.make_identity(nc, ident[:, :])

        cov_t = cons.tile([P, 6, TOTF], BF16, tag="cov", name="cov_t")
        mean_t = cons.tile([P, 3, TOTF], F32, tag="mean", name="mean_t")

        def eng_of(code):
            return nc.gpsimd if code == "P" else nc.vector

        # ---------------- phase A (generator; yields at segment breaks) ----
        def phase_a_chunk(c):
            cfg = A_ENG[c]
            bsl = slice(c * BPCH, (c + 1) * BPCH)
            sl = slice(c * CHF, (c + 1) * CHF)

            def tt(group, fn, *args):
                getattr(eng_of(cfg[group]), f"tensor_{fn}")(*args)

            def load_plane(tag, src, eng=None):
                t = io.tile([P, CHF], F32, tag=tag, name=tag)
                (eng or nc.sync).dma_start(
                    out=t[:, :].rearrange("p (b f) -> p b f", f=FPB),
                    in_=src.rearrange("b (p f) -> p b f", p=P))
                return t
            sc_eng = ({"SP": nc.sync, "A": nc.scalar, "P": nc.gpsimd}[
                _knob("K_SC_ENG", "SP")] if c == 0 else None)

            r = load_plane("rot_r", rot_s[bsl, 0, :])
            x = load_plane("rot_x", rot_s[bsl, 1, :])
            y = load_plane("rot_y", rot_s[bsl, 2, :])
            z = load_plane("rot_z", rot_s[bsl, 3, :])
            s0 = load_plane("sc_0", scale_s[bsl, 0, :], sc_eng)
            s1 = load_plane("sc_1", scale_s[bsl, 1, :], sc_eng)
            s2 = load_plane("sc_2", scale_s[bsl, 2, :], sc_eng)

            def Tf(tag):
                return pa.tile([P, CHF], F32, tag=tag, name=tag)

            def Tb(tag):
                return pa.tile([P, CHF], BF16, tag=tag, name=tag)

            # Act: interleaved doubles/squares for earliest availability
            r2, x2, y2, z2 = Tf("fa"), Tf("fb"), Tf("fc"), Tf("fd")
            xt, yt, zt = Tf("fe"), Tf("ff_"), Tf("fg")
            nc.scalar.square(r2, r)
            nc.scalar.activation(xt, x, AF.Copy, scale=2.0)
            nc.scalar.square(x2, x)
            nc.scalar.activation(yt, y, AF.Copy, scale=2.0)
            nc.scalar.square(y2, y)
            nc.scalar.activation(zt, z, AF.Copy, scale=2.0)
            nc.scalar.square(z2, z)
            s0q, s1q, s2q = Tb("s0q"), Tb("s1q"), Tb("s2q")
            nc.scalar.square(s0q, s0)
            nc.scalar.square(s1q, s1)
            nc.scalar.square(s2q, s2)

            # pair products first: they head the DVE stream in chunk 0
            xy2, xz2, yz2 = Tf("pa_"), Tf("pb_"), Tf("pc_")
            rx2, ry2, rz2 = Tf("pd_"), Tf("pe_"), Tf("pf_")
            tt("prods", "mul", xy2, xt, y)
            tt("prods", "mul", xz2, xt, z)
            tt("prods", "mul", yz2, yt, z)
            tt("prods", "mul", rx2, r, xt)
            tt("prods", "mul", ry2, r, yt)
            tt("prods", "mul", rz2, r, zt)

            # sums (6) + n2 (f32)
            aa, bb, cc = Tf("fh"), Tf("fi"), Tf("fj")
            dd, ee, ff = Tf("fk"), Tf("fl"), Tf("fm")
            tt("sums", "add", aa, r2, x2)
            tt("sums", "add", bb, y2, z2)
            tt("sums", "add", cc, r2, y2)
            tt("sums", "add", dd, x2, z2)
            tt("sums", "add", ee, r2, z2)
            tt("sums", "add", ff, x2, y2)
            n2 = Tf("fn")
            nc.vector.tensor_add(n2, aa, bb)
            rr = Tf("fo")
            nc.vector.reciprocal(rr, n2)
            iv = Tb("iv")
            nc.scalar.square(iv, rr)

            yield

            # J planes (f32 compute, bf16 store) — reuse dead f32 slots
            jtag = {0: "fa", 4: "fb", 8: "fc",
                    1: "fd", 3: "fh", 2: "fi", 6: "fj", 5: "fk", 7: "fl"}
            J = {}
            for idx, (u, v_) in zip((0, 4, 8), ((aa, bb), (cc, dd), (ee, ff))):
                J[idx] = Tb(jtag[idx])
                tt("jdiag", "sub", J[idx], u, v_)

            for idx, fn, (u, v_) in (
                    (1, "sub", (xy2, rz2)), (3, "add", (xy2, rz2)),
                    (2, "add", (xz2, ry2)), (6, "sub", (xz2, ry2)),
                    (5, "sub", (yz2, rx2)), (7, "add", (yz2, rx2))):
                J[idx] = Tb(jtag[idx])
                tt("joff", fn, J[idx], u, v_)

            yield

            b0, b1 = Tb("b0"), Tb("b1")
            tt("b", "sub", b0, s0q, s2q)
            tt("b", "sub", b1, s1q, s2q)
            bt0, bt1 = Tb("bt0"), Tb("bt1")
            tt("bt", "mul", bt0, b0, iv)
            tt("bt", "mul", bt1, b1, iv)

            wtag = ["s0q", "s1q", "b0", "b1", "fm", "fn"]
            w = {}
            for i in range(3):
                for k in range(2):
                    w[(i, k)] = Tb(wtag[i * 2 + k])
                    tt("w", "mul", w[(i, k)], bt0 if k == 0 else bt1,
                       J[i * 3 + k])

            yield

            # cov planes: cov_ij = w_i0*J_j0 + w_i1*J_j1 (+ s2q if i==j)
            for e, (i, j) in enumerate([(0, 0), (1, 1), (2, 2),
                                        (0, 1), (0, 2), (1, 2)]):
                m1 = pa.tile([P, CHF], BF16, tag=("iv", "bt0")[e % 2],
                             name="am1")
                m2 = pa.tile([P, CHF], BF16, tag=("bt1", "am2x")[e % 2],
                             name="am2")
                tt("m", "mul", m1, w[(i, 0)], J[j * 3 + 0])
                tt("m", "mul", m2, w[(i, 1)], J[j * 3 + 1])
                dst = cov_t[:, PLANE[(i, j)], sl]
                if i == j:
                    tt("madd", "add", dst, m1, m2)
                    tt("madd", "add", dst, dst, s2q)
                else:
                    tt("madd", "add", dst, m1, m2)

            yield

        # ---------------- phase B ------------------------------------------
        e_ts, cx_ts = {}, {}

        def prep(b):
            """eps DMA + split (Act), cov s-expansion (Act/Pool) for batch b."""
            bsl = slice(b * FPB, (b + 1) * FPB)
            e_t = ep.tile([P, 3, FPB * S], BF16, tag="e", name="e_t")
            e_ts[b] = e_t
            EPS_ENG = _knob("K_EPS_ENG", {})
            for q in range(NQ):
                eq = epsp.tile([P, FQ * S * 3], F32, tag="epsq", name="eq")
                engs = EPS_ENG.get(str(b), ["SP"] * NQ)
                deng = {"SP": nc.sync, "A": nc.scalar,
                        "P": nc.gpsimd}[engs[q % len(engs)]]
                deng.dma_start(
                    out=eq[:, :],
                    in_=eps_s[b * N:(b + 1) * N, :, :].rearrange(
                        "(p f) s i -> p (f s i)", p=P)[
                        :, q * FQ * S * 3:(q + 1) * FQ * S * 3])
                if (q in _knob("K_POOL_ESPLIT_Q", []) or
                        b in _knob("K_POOL_ESPLIT_B", [])):
                    nc.gpsimd.tensor_copy(
                        e_t[:, :, q * FQ * S:(q + 1) * FQ * S],
                        eq[:, :].rearrange("p (f s i) -> p i (f s)", s=S, i=3))
                elif b in _knob("K_DVE_ESPLIT", [3, 4, 5, 6, 7]):
                    nc.vector.tensor_copy(
                        e_t[:, :, q * FQ * S:(q + 1) * FQ * S],
                        eq[:, :].rearrange("p (f s i) -> p i (f s)", s=S, i=3))
                else:
                    nc.scalar.activation(
                        e_t[:, :, q * FQ * S:(q + 1) * FQ * S],
                        eq[:, :].rearrange("p (f s i) -> p i (f s)", s=S, i=3),
                        AF.Copy)

            cx = cxp.tile([P, 6, FPB, S], BF16, tag="cx", name="cx")
            cx_ts[b] = cx
            nd = DVE_CEXP.get(b, 0)
            for u in range(POOL_CEXP):
                nc.gpsimd.tensor_copy(cx[:, u], _bcast_s(cov_t[:, u, bsl]))
            for u in range(POOL_CEXP, POOL_CEXP + nd):
                nc.vector.tensor_copy(cx[:, u], _bcast_s(cov_t[:, u, bsl]))
            u0 = POOL_CEXP + nd
            while u0 < 6:
                u1 = min(u0 + 2, 6)   # pairs: amortize init, keep granularity
                nc.scalar.activation(
                    cx[:, u0:u1], _bcast_s(cov_t[:, u0:u1, bsl]), AF.Copy)
                u0 = u1

        out_ts = {}

        OUT_ENG = _knob("K_OUT_ENG", {"6": ["SP", "A"], "7": ["SP", "A"]})

        def emit_out(b):
            out_t, nh = out_ts.pop(b)
            engs = OUT_ENG.get(str(b), ["SP"] * nh)
            ob = out_s[b, :, :].rearrange("j (p f) -> p j f", p=P)
            fh = FPB * S // nh
            for h in range(nh):
                fsl = slice(h * fh, (h + 1) * fh)
                eng = {"SP": nc.sync, "A": nc.scalar,
                       "P": nc.gpsimd}[engs[h % len(engs)]]
                eng.dma_start(out=ob[:, :, fsl], in_=out_t[:, :, fsl])

        def compute(b):
            e_t, cx = e_ts.pop(b), cx_ts.pop(b)

            m = [mp.tile([P, 3, FPB * S], BF16, tag=f"m{i}", name=f"m{i}")
                 for i in range(3)]
            v = vp.tile([P, 3, FPB * S], BF16, tag="v", name="v")
            out_t = outp.tile([P, 3, FPB * S], F32, tag="out", name="out_t")
            nh = max(NH.get(b, 1), 2)
            out_ts[b] = (out_t, nh)
            fh = FPB // nh
            for h in range(nh):
                fsl = slice(h * fh * S, (h + 1) * fh * S)      # (f s) slice
                csl = slice(h * fh, (h + 1) * fh)              # f slice
                bsl = slice(b * FPB + h * fh, b * FPB + (h + 1) * fh)
                MORDER = _knob("K_MORDER", [[0, 0], [1, 1], [0, 1], [1, 0],
                                            [2, 2], [0, 2], [1, 2], [2, 0],
                                            [2, 1]])
                for i, j in MORDER:
                    eng = (nc.gpsimd if
                           (i == 2 and b in POOL_MUL_BATCHES) or
                           (i == 2 and j == 2 and
                            b in _knob("K_POOL_MUL1", []))
                           else nc.vector)
                    eng.tensor_mul(
                        m[i][:, j, fsl],
                        e_t[:, i, fsl],
                        cx[:, PLANE[(i, j)], csl].rearrange(
                            "p f s -> p (f s)"))

                # PE: v_psum = m0 + m1 + m2 via identity-matmul accumulation
                # (one PSUM bank per j-plane; 512 f32 each)
                ps = psp.tile([P, 3, fh * S], F32, tag="ps", name="ps")
                for j in range(3):
                    for i in range(3):
                        nc.tensor.matmul(
                            ps[:, j, :], ident[:, :], m[i][:, j, fsl],
                            start=(i == 0), stop=(i == 2))
                vh = v[:, :, fsl]
                if b in _knob("K_FUSE_CLIP", []):
                    # fused PSUM drain + clip on DVE (1x, but one hop less)
                    nc.vector.tensor_scalar(
                        out=vh, in0=ps[:, :, :],
                        scalar1=1.0, scalar2=-1.0, op0=OP.min, op1=OP.max)
                else:
                    # Act: PSUM -> SBUF, f32 -> bf16
                    nc.scalar.activation(vh, ps[:, :, :], AF.Copy)
                    # DVE: clip in place (bf16 tensor_scalar, 4x)
                    nc.vector.tensor_scalar(
                        out=vh, in0=vh,
                        scalar1=1.0, scalar2=-1.0, op0=OP.min, op1=OP.max)
                # + mean, f32 out (Pool unless overridden)
                o4 = out_t[:, :, fsl].rearrange("p j (f s) -> p j f s", s=S)
                c4 = vh.rearrange("p j (f s) -> p j f s", s=S)
                feng = (nc.vector if b in _knob("K_DVE_FINAL", [])
                        or (b in _knob("K_ALT_FINAL", []) and h % 2)
                        else nc.gpsimd)
                feng.tensor_add(o4, c4, _bcast_s(mean_t[:, :, bsl]))

        # ---------------- emission schedule --------------------------------
        a0 = phase_a_chunk(0)
        for _ in a0:
            pass
        prep(0)
        # mean loads on the Act queue (v1 cost model: DMA occupies the
        # issuing engine, so SP and Act transfers run concurrently)
        mengs = _knob("K_MEAN_ENG", ["P", "P", "P"])
        for j in range(3):
            meng = {"SP": nc.sync, "A": nc.scalar,
                    "P": nc.gpsimd}[mengs[j % len(mengs)]]
            meng.dma_start(
                out=mean_t[:, j, :].rearrange("p (b f) -> p b f", f=FPB),
                in_=mean_s[:, j, :].rearrange("b (p f) -> p b f", p=P))
        odl = _knob("K_OUTDELAY", 2)
        pend = []

        def flush(keep):
            while len(pend) > keep:
                emit_out(pend.pop(0))

        a1 = phase_a_chunk(1)
        next(a1)
        prep(1)
        next(a1)
        compute(0)
        pend.append(0)
        prep(2)
        next(a1)
        flush(odl)
        compute(1)
        pend.append(1)
        prep(3)
        for _ in a1:
            pass
        for b in range(2, BPC):
            if b + 2 < BPC:
                prep(b + 2)
            flush(odl)
            compute(b)
            pend.append(b)
        flush(0)

    if split:
        split_excess_waits(nc)
    return nc


_NC = None


def kernel(mean, scale, rot, epsilon, num_samples):
    global _NC
    assert int(num_samples) == S
    mean = np.asarray(mean, dtype=np.float32)
    scale = np.asarray(scale, dtype=np.float32)
    rot = np.asarray(rot, dtype=np.float32)
    epsilon = np.asarray(epsilon, dtype=np.float32)
    if _NC is None:
        _NC = build()
    from concourse.bass_utils import run_bass_kernel_spmd
    in_maps = []
    for c in range(NCORES):
        bs = slice(c * BPC, (c + 1) * BPC)
        in_maps.append({
            "mean": np.ascontiguousarray(mean[bs]),
            "scale": np.ascontiguousarray(scale[bs]),
            "rot": np.ascontiguousarray(rot[bs]),
            "eps": np.ascontiguousarray(epsilon[c * BPC * N:(c + 1) * BPC * N]),
        })
    res = run_bass_kernel_spmd(_NC, in_maps, core_ids=list(range(NCORES)))
    return np.concatenate([res.results[i]["out"] for i in range(NCORES)], axis=0)
